# revision 1
# baseline (speedup 1.0000x reference)
"""Causal multi-head attention (fused QKV projection + attention) on 8 TRN2 cores.

Sharding: data-parallel over batch (2) x tensor-parallel over head groups (4).
Each core computes 4 heads of one batch element end-to-end; no collectives.

Device kernel design (per core):
  - Host feeds x[b] pre-transposed (xT [1024, 2048]) so every matmul contracts
    over the partition dimension without any on-device transposes.
  - QKV projection:
      q,k produced TRANSPOSED ([feature, t]): psum = W_col_chunk.T @ xT_chunk,
        accumulated over 8 k-chunks. Features packed so each head's 64-dim
        q/k lands at base partition 0 or 64 -> the two heads of a pair run
        CONCURRENTLY on the PE via row tiling (K=64 each).
      v produced NATURAL ([t, feature]): psum = xT_chunk.T @ Wv_chunk, plus a
        K=1 ones-outer-product matmul to add the bias. v stored as [t, h, 65]
        with a constant 1.0 in column 64 (V_aug).
  - Attention per (pair, tq-chunk c of 512, key-block j of 128):
      scoresT[j-block, tq] = kT.T @ qT  (both heads into one 2-bank psum tile)
      expT = exp(scoresT / 8) on ScalarE (single instr for both heads);
      causal handled by slicing columns to [128r:512] for diagonal-crossing
      blocks + one 0/1 triangular mask multiply on the diagonal 128x128.
      psum_o[65, tq] += V_aug.T @ expT  -> rows 0..63 = unnormalized out^T,
      row 64 = softmax denominators (ones-column trick).
  - Normalize: reciprocal of row 64, gpsimd partition_broadcast, multiply,
    DMA out^T [64, 512] per (head, chunk) to DRAM.
"""

import sys

if "/opt/trn_rl_repo" not in sys.path:
    sys.path.insert(0, "/opt/trn_rl_repo")

import numpy as np

B = 2
T = 2048
D = 1024
H = 16
HD = 64
NCORES = 8
GROUPS = 4  # head groups (tensor-parallel)
HPC = 4  # heads per core
P = 128
TCH = 512  # tq chunk width
NKC = D // P  # 8 contraction chunks
NTB = T // P  # 16 key blocks
NTC = T // TCH  # 4 tq chunks


def build_nc(external_io=True, loops=1, do_proj=True, do_attn=True, has_bias=True, interleave=False):
    import concourse.mybir as mybir
    from concourse import bacc
    from concourse.tile import TileContext

    f32 = mybir.dt.float32
    f32r = mybir.dt.float32r
    Exp = mybir.ActivationFunctionType.Exp

    # fp32 matmuls run at 4 cycles/row on the PE; float32r (same 4-byte data,
    # reduced-precision single-pass mode) runs at 1 cycle/row for moving dims
    # >= 256 — so every tensor feeding a matmul is typed float32r.
    nc = bacc.Bacc(None)
    if external_io:
        xT_d = nc.dram_tensor("xT", [D, T], f32r, kind="ExternalInput")
        wqk_d = nc.dram_tensor("wqk", [D, 4 * P], f32r, kind="ExternalInput")
        bqk_d = nc.dram_tensor("bqk", [4 * P], f32, kind="ExternalInput")
        wv_d = nc.dram_tensor("wv", [D, HPC * HD], f32r, kind="ExternalInput")
        bv_d = nc.dram_tensor("bv", [HPC * HD], f32, kind="ExternalInput")
        mask_d = nc.dram_tensor("mask", [P, 2 * P], f32, kind="ExternalInput")
        onescol_d = nc.dram_tensor(
            "onescol", [P, NTB, HPC, 1], f32r, kind="ExternalInput"
        )
        out_d = nc.dram_tensor("out", [HPC, HD, T], f32, kind="ExternalOutput")
    else:
        # timing-only variant: real I/O lives in internal DRAM (uninitialized
        # garbage — identical instruction stream and timing, no per-call
        # host<->device traffic). Tiny external tensors keep the PJRT
        # interface alive.
        xT_d = nc.dram_tensor("xT", [D, T], f32r)
        wqk_d = nc.dram_tensor("wqk", [D, 4 * P], f32r)
        bqk_d = nc.dram_tensor("bqk", [4 * P], f32)
        wv_d = nc.dram_tensor("wv", [D, HPC * HD], f32r)
        bv_d = nc.dram_tensor("bv", [HPC * HD], f32)
        mask_d = nc.dram_tensor("mask", [P, 2 * P], f32, kind="ExternalInput")
        onescol_d = nc.dram_tensor("onescol", [P, NTB, HPC, 1], f32r)
        out_d = nc.dram_tensor("out", [HPC, HD, T], f32)
        tiny_out_d = nc.dram_tensor("tiny", [P, P], f32, kind="ExternalOutput")

    with TileContext(nc) as tc:
        with (
            tc.tile_pool(name="const", bufs=1) as cpool,
            tc.tile_pool(name="work", bufs=3) as wpool,
            tc.tile_pool(name="opool", bufs=2) as opool,
            tc.tile_pool(name="psA", bufs=2, space="PSUM") as psA,
            tc.tile_pool(name="psB", bufs=4, space="PSUM") as psB,
            tc.tile_pool(name="dramp", bufs=3, space="DRAM") as dram_pool,
        ):
          for _rep in range(loops):
            xT_sb = cpool.tile([P, NKC, T], f32r)
            qkT_sb = cpool.tile([P, 4, T], f32r)
            v_sb = cpool.tile([P, NTB, HPC, HD + 1], f32r)
            wqk_sb = cpool.tile([P, NKC, 4 * P], f32r)
            wv_sb = cpool.tile([P, NKC, HPC * HD], f32r)
            bqk_sb = cpool.tile([P, 4], f32)
            bv_sb = cpool.tile([1, HPC * HD], f32)
            mask_sb = cpool.tile([P, 2 * P], f32)

            nc.sync.dma_start(v_sb[:, :, :, HD : HD + 1], onescol_d[:])
            if not do_proj and do_attn:
                # timing-only: attention reads need a writer for allocation
                nc.gpsimd.memset(qkT_sb[:], 0.0)
                nc.gpsimd.memset(v_sb[:, :, :, 0:HD], 0.0)
            nc.sync.dma_start(mask_sb[:], mask_d[:])
            nc.sync.dma_start(bv_sb[:], bv_d[None, :])
            nc.sync.dma_start(bqk_sb[:], bqk_d.rearrange("(n p) -> p n", p=P))
            wqk_view = wqk_d.rearrange("(ko p) n -> p ko n", p=P)
            wv_view = wv_d.rearrange("(ko p) n -> p ko n", p=P)
            xT_view = xT_d.rearrange("(ko p) t -> p ko t", p=P)
            # per-k-chunk DMAs: the first projection psum-group consumes all 8
            # chunks within ~2us of PE time, so fine-grained transfers let it
            # start as soon as the tail chunk lands instead of after one big
            # serialized transfer per tensor
            for kc in range(NKC):
                nc.sync.dma_start(wqk_sb[:, kc, :], wqk_view[:, kc, :])
                nc.sync.dma_start(wv_sb[:, kc, :], wv_view[:, kc, :])
                nc.sync.dma_start(xT_sb[:, kc, :], xT_view[:, kc, :])

            # ---------------- phase bodies ----------------
            def proj_qk(pair, tci):
                tsl = slice(tci * TCH, (tci + 1) * TCH)
                for n in (pair, 2 + pair):
                    pq = psB.tile([P, TCH], f32, tag="acc", name="pq")
                    for kc in range(NKC):
                        nc.tensor.matmul(
                            pq[:],
                            wqk_sb[:, kc, n * P : (n + 1) * P],
                            xT_sb[:, kc, tsl],
                            start=(kc == 0),
                            stop=(kc == NKC - 1),
                        )
                    if has_bias:
                        nc.vector.tensor_add(
                            qkT_sb[:, n, tsl],
                            pq[:],
                            bqk_sb[:, n : n + 1].to_broadcast((P, TCH)),
                        )
                    else:
                        nc.vector.tensor_copy(qkT_sb[:, n, tsl], pq[:])

            def proj_v(tci):
                for tb in range(tci * 4, tci * 4 + 4):
                    pv = psB.tile([P, HPC * HD], f32, tag="acc", name="pv")
                    for kc in range(NKC):
                        nc.tensor.matmul(
                            pv[:],
                            xT_sb[:, kc, tb * P : (tb + 1) * P],
                            wv_sb[:, kc, :],
                            start=(kc == 0),
                            stop=(kc == NKC - 1 and not has_bias),
                        )
                    if has_bias:
                        nc.tensor.matmul(
                            pv[:],
                            mask_sb[0:1, P : 2 * P],
                            bv_sb[:1, :],
                            start=False,
                            stop=True,
                        )
                    nc.vector.tensor_copy(
                        v_sb[:, tb, :, 0:HD],
                        pv[:].rearrange("p (h d) -> p h d", d=HD),
                    )

            def attn_chunk(pair, ci):
                qn, kn = pair, 2 + pair
                jmax = 4 * ci + 3
                po = [
                    psB.tile([HD + 1, TCH], f32, tag="acc", name=f"po{hip}")
                    for hip in range(2)
                ]

                def qk_exp(j):
                    r = j - 4 * ci
                    # r=3 widened to N=256 so fp32r stays on the 1-cycle/row
                    # path (N<256 falls back to 4 cycles/row)
                    col0 = min(P * r, 2 * P) if r > 0 else 0
                    ps = psA.tile([P, 2, TCH], f32, tag="sc")
                    et = wpool.tile([P, 2, TCH], f32r, tag="expt")
                    for hip in range(2):
                        base = 64 * hip
                        nc.tensor.matmul(
                            ps[:, hip, col0:],
                            qkT_sb[base : base + 64, kn, j * P : (j + 1) * P],
                            qkT_sb[
                                base : base + 64,
                                qn,
                                ci * TCH + col0 : (ci + 1) * TCH,
                            ],
                            start=True,
                            stop=True,
                        )
                    nc.scalar.activation(
                        et[:, :, col0:], ps[:, :, col0:], Exp, scale=0.125
                    )
                    if r >= 0:
                        if r == 3:  # widened: zero cols [256:384] + triangle
                            nc.vector.tensor_mul(
                                et[:, :, 2 * P : 4 * P],
                                et[:, :, 2 * P : 4 * P],
                                mask_sb[:, None, :].to_broadcast((P, 2, 2 * P)),
                            )
                        else:
                            nc.vector.tensor_mul(
                                et[:, :, col0 : col0 + P],
                                et[:, :, col0 : col0 + P],
                                mask_sb[:, None, P : 2 * P].to_broadcast((P, 2, P)),
                            )
                    return et, col0

                def av(j, et, col0):
                    for hip in range(2):
                        h = 2 * pair + hip
                        nc.tensor.matmul(
                            po[hip][:, col0:],
                            v_sb[:, j, h, :],
                            et[:, hip, col0:],
                            start=(j == 0),
                            stop=(j == jmax),
                        )

                prev = None
                for j in range(jmax + 1):
                    cur = qk_exp(j)
                    if prev is not None:
                        av(j - 1, *prev)
                    prev = cur
                av(jmax, *prev)

                for hip in range(2):
                    h = 2 * pair + hip
                    # one DVE copy frees the PSUM bank immediately; the rest of
                    # the normalization runs from SBUF off the critical path
                    pou = opool.tile([HD + 1, TCH], f32, tag="pou")
                    nc.vector.tensor_copy(pou[:], po[hip][:])
                    recip = opool.tile([1, TCH], f32, tag="recip")
                    nc.vector.reciprocal(recip[:], pou[HD : HD + 1, :])
                    scr = dram_pool.tile([1, TCH], f32, tag="scr")
                    nc.sync.dma_start(scr[:], recip[:])
                    rbc = opool.tile([HD, TCH], f32, tag="rbc")
                    nc.sync.dma_start(rbc[:], scr[:].to_broadcast((HD, TCH)))
                    osb = opool.tile([HD, TCH], f32, tag="osb")
                    nc.vector.tensor_mul(osb[:], pou[0:HD, :], rbc[:])
                    nc.sync.dma_start(
                        out_d[h, :, ci * TCH : (ci + 1) * TCH], osb[:]
                    )

            # ---------------- schedule ----------------
            # Interleave pair-1 projection between pair-0 attention chunks so
            # ScalarE (exp) never idles after its first chunk and the PE fills
            # its ACT-wait gaps with projection matmuls.
            if do_proj and do_attn:
                if interleave:
                    proj_qk(0, 0)
                    proj_v(0)
                    for ci in range(NTC):
                        attn_chunk(0, ci)
                        if ci + 1 < NTC:
                            proj_qk(0, ci + 1)
                            proj_v(ci + 1)
                        proj_qk(1, ci)
                    for ci in range(NTC):
                        attn_chunk(1, ci)
                else:
                    for tci in range(NTC):
                        proj_qk(0, tci)
                        proj_qk(1, tci)
                        proj_v(tci)
                    for pair in range(2):
                        for ci in range(NTC):
                            attn_chunk(pair, ci)
            elif do_proj:
                for tci in range(NTC):
                    proj_qk(0, tci)
                    proj_qk(1, tci)
                    proj_v(tci)
            elif do_attn:
                for pair in range(2):
                    for ci in range(NTC):
                        attn_chunk(pair, ci)
            if not external_io:
                nc.sync.dma_start(tiny_out_d[:], mask_sb[:, 0:P])
    if not nc.is_finalized():
        nc.finalize()
    return nc


def make_in_maps(x, W, b):
    x = np.asarray(x, np.float32)
    W = np.asarray(W, np.float32)
    b = np.asarray(b, np.float32)
    tri = (np.arange(P)[:, None] <= np.arange(P)[None, :]).astype(np.float32)
    mask01 = np.concatenate([np.zeros((P, P), np.float32), tri], axis=1)
    in_maps = []
    for core in range(NCORES):
        bidx, g = divmod(core, GROUPS)
        xT = np.ascontiguousarray(x[bidx].T)
        cols = np.empty(4 * P, np.int64)
        for n in range(4):
            qk, pairi = divmod(n, 2)
            for p in range(P):
                hl = 2 * pairi + p // 64
                cols[n * P + p] = qk * D + (HPC * g + hl) * HD + (p % 64)
        sl = slice(2 * D + g * HPC * HD, 2 * D + (g + 1) * HPC * HD)
        in_maps.append(
            {
                "xT": xT,
                "onescol": np.ones((P, NTB, HPC, 1), np.float32),
                "wqk": np.ascontiguousarray(W[:, cols]),
                "bqk": np.ascontiguousarray(b[cols]),
                "wv": np.ascontiguousarray(W[:, sl]),
                "bv": np.ascontiguousarray(b[sl]),
                "mask": mask01,
            }
        )
    return in_maps


def assemble_output(per_core_out):
    O = np.empty((B, H, HD, T), np.float32)
    for core in range(NCORES):
        bidx, g = divmod(core, GROUPS)
        O[bidx, g * HPC : (g + 1) * HPC] = per_core_out[core]
    return np.ascontiguousarray(O.transpose(0, 3, 1, 2).reshape(B, T, H * HD))


def run(x, W_qkv, b_qkv, trace=False):
    from concourse.bass_utils import run_bass_kernel_spmd

    nc = build_nc(has_bias=bool(np.any(np.asarray(b_qkv))))
    in_maps = make_in_maps(x, W_qkv, b_qkv)
    res = run_bass_kernel_spmd(
        nc, in_maps, list(range(NCORES)), trace=trace
    )
    out = assemble_output([res.results[i]["out"] for i in range(NCORES)])
    return out, res


def kernel(x, W_qkv, b_qkv):
    out, _ = run(x, W_qkv, b_qkv, trace=False)
    return out



# revision 4
# speedup vs baseline: 1.5356x; 1.5356x over previous
"""Causal multi-head attention (fused QKV projection + attention) on 8 TRN2 cores.

Sharding: data-parallel over batch (2) x tensor-parallel over head groups (4).
Each core computes 4 heads of one batch element end-to-end; no collectives.

Device kernel design (per core), all matmul operands bf16 (1 cycle/row on the
PE at any moving width; rel err ~5e-3 vs the 2e-2 gate):
  - Host feeds x[b] pre-transposed (xT [1024, 2048], bf16) so every matmul
    contracts over the partition dimension without on-device transposes.
  - DMA granularity: wqk first, then xT in (tci, kc) blocks of [128, 512] so
    the tci=0 projection starts after ~2MB instead of the full input.
  - QKV projection:
      q,k produced TRANSPOSED ([feature, t]): psum = W_col_chunk.T @ xT_chunk,
        accumulated over 8 k-chunks. Features packed so each head's 64-dim
        q/k lands at base partition 0 or 64 -> the two heads of a pair run
        CONCURRENTLY on the PE via row tiling (K=64 each).
      v produced NATURAL ([t, feature]): psum = xT_chunk.T @ Wv_chunk; v
        stored bf16 as [t, h, 65] with constant 1.0 in column 64 (V_aug).
  - Attention per (pair, tq-chunk c of 512, key-block j of 128):
      scoresT[j-block, tq] = kT.T @ qT  (both heads into one 2-bank psum tile)
      expT = exp(scoresT / 8) on ScalarE -> bf16; causal = one 0/1 triangular
      mask multiply on the 128 diagonal columns (DVE 2x mode on bf16).
      psum_o[65, tq] += V_aug.T @ expT  -> rows 0..63 = unnormalized out^T,
      row 64 = softmax denominators (ones-column trick).
  - Normalize: DVE copy to SBUF (frees psum), DVE reciprocal of row 64,
    gpsimd partition_broadcast, DVE multiply, DMA out^T [64, 512] to DRAM.
  - Schedule: proj(tci=0) first, then per chunk ci: attention for both pairs
    emitted before proj(tci=ci+1) -- the Tile scheduler fills PE gaps during
    ScalarE-bound exp stretches with next-chunk projection matmuls.
"""

import sys

if "/opt/trn_rl_repo" not in sys.path:
    sys.path.insert(0, "/opt/trn_rl_repo")

import numpy as np

B = 2
T = 2048
D = 1024
H = 16
HD = 64
NCORES = 8
GROUPS = 4  # head groups (tensor-parallel)
HPC = 4  # heads per core
P = 128
TCH = 512  # tq chunk width
NKC = D // P  # 8 contraction chunks
NTB = T // P  # 16 key blocks
NTC = T // TCH  # 4 tq chunks


def build_nc(external_io=True, loops=1, do_proj=True, do_attn=True, has_bias=True, interleave=True):
    import concourse.mybir as mybir
    from concourse import bacc
    from concourse.tile import TileContext

    f32 = mybir.dt.float32
    bf16 = mybir.dt.bfloat16
    Exp = mybir.ActivationFunctionType.Exp

    nc = bacc.Bacc(None)
    if external_io:
        xT_d = nc.dram_tensor("xT", [D, T], bf16, kind="ExternalInput")
        wqk_d = nc.dram_tensor("wqk", [D, 4 * P], bf16, kind="ExternalInput")
        bqk_d = nc.dram_tensor("bqk", [4 * P], f32, kind="ExternalInput")
        wv_d = nc.dram_tensor("wv", [D, HPC * HD], bf16, kind="ExternalInput")
        bv_d = nc.dram_tensor("bv", [HPC * HD], bf16, kind="ExternalInput")
        mask_d = nc.dram_tensor("mask", [P, 2 * P], bf16, kind="ExternalInput")
        onescol_d = nc.dram_tensor(
            "onescol", [P, NTB, HPC, 1], bf16, kind="ExternalInput"
        )
        out_d = nc.dram_tensor("out", [HPC, HD, T], f32, kind="ExternalOutput")
    else:
        # timing-only variant: real I/O lives in internal DRAM (uninitialized
        # garbage -- identical instruction stream and timing, no per-call
        # host<->device traffic). Tiny external tensors keep the PJRT
        # interface alive.
        xT_d = nc.dram_tensor("xT", [D, T], bf16)
        wqk_d = nc.dram_tensor("wqk", [D, 4 * P], bf16)
        bqk_d = nc.dram_tensor("bqk", [4 * P], f32)
        wv_d = nc.dram_tensor("wv", [D, HPC * HD], bf16)
        bv_d = nc.dram_tensor("bv", [HPC * HD], bf16)
        mask_d = nc.dram_tensor("mask", [P, 2 * P], bf16, kind="ExternalInput")
        onescol_d = nc.dram_tensor("onescol", [P, NTB, HPC, 1], bf16)
        out_d = nc.dram_tensor("out", [HPC, HD, T], f32)
        tiny_out_d = nc.dram_tensor("tiny", [P, P], bf16, kind="ExternalOutput")

    with TileContext(nc) as tc:
        with (
            tc.tile_pool(name="const", bufs=1) as cpool,
            tc.tile_pool(name="work", bufs=3) as wpool,
            tc.tile_pool(name="opool", bufs=2) as opool,
            tc.tile_pool(name="psA", bufs=2, space="PSUM") as psA,
            tc.tile_pool(name="psO", bufs=1, space="PSUM") as psO,
            tc.tile_pool(name="psP", bufs=2, space="PSUM") as psP,
        ):
          # loops>1 (timing builds): body wrapped in a hardware For_i loop so
          # a single dispatch amortizes the ~100ms axon round-trip over many
          # reps; program size stays constant.
          def _body():
            xT_sb = cpool.tile([P, NKC, T], bf16)
            qkT_sb = cpool.tile([P, 4, T], bf16)
            v_sb = cpool.tile([P, NTB, HPC, HD + 1], bf16)
            wqk_sb = cpool.tile([P, NKC, 4 * P], bf16)
            wv_sb = cpool.tile([P, NKC, HPC * HD], bf16)
            bqk_sb = cpool.tile([P, 4], f32)
            bv_sb = cpool.tile([1, HPC * HD], bf16)
            mask_sb = cpool.tile([P, 2 * P], bf16)

            nc.sync.dma_start(v_sb[:, :, :, HD : HD + 1], onescol_d[:])
            if not do_proj and do_attn:
                # timing-only: attention reads need a writer for allocation
                nc.gpsimd.memset(qkT_sb[:], 0.0)
                nc.gpsimd.memset(v_sb[:, :, :, 0:HD], 0.0)
            nc.sync.dma_start(mask_sb[:], mask_d[:])
            nc.sync.dma_start(bv_sb[:], bv_d[None, :])
            nc.sync.dma_start(bqk_sb[:], bqk_d.rearrange("(n p) -> p n", p=P))
            wqk_view = wqk_d.rearrange("(ko p) n -> p ko n", p=P)
            wv_view = wv_d.rearrange("(ko p) n -> p ko n", p=P)
            xT_view = xT_d.rearrange("(ko p) t -> p ko t", p=P)
            # load order: wqk, xT[tci=0], wv, xT[tci=1..3] -- the tci=0
            # projection (and attention chunk 0 behind it) becomes runnable
            # after ~2.5MB of the 5.5MB input stream
            for kc in range(NKC):
                nc.sync.dma_start(wqk_sb[:, kc, :], wqk_view[:, kc, :])
            ts0 = slice(0, TCH)
            for kc in range(NKC):
                nc.sync.dma_start(xT_sb[:, kc, ts0], xT_view[:, kc, ts0])
            for kc in range(NKC):
                nc.sync.dma_start(wv_sb[:, kc, :], wv_view[:, kc, :])
            for tci in range(1, NTC):
                tsl = slice(tci * TCH, (tci + 1) * TCH)
                for kc in range(NKC):
                    nc.sync.dma_start(xT_sb[:, kc, tsl], xT_view[:, kc, tsl])

            # ---------------- phase bodies ----------------
            def proj_qk(pair, tci):
                tsl = slice(tci * TCH, (tci + 1) * TCH)
                for n in (pair, 2 + pair):
                    pq = psP.tile([P, TCH], f32, tag="pp", name="pq")
                    for kc in range(NKC):
                        nc.tensor.matmul(
                            pq[:],
                            wqk_sb[:, kc, n * P : (n + 1) * P],
                            xT_sb[:, kc, tsl],
                            start=(kc == 0),
                            stop=(kc == NKC - 1),
                        )
                    if has_bias:
                        nc.vector.tensor_add(
                            qkT_sb[:, n, tsl],
                            pq[:],
                            bqk_sb[:, n : n + 1].to_broadcast((P, TCH)),
                        )
                    else:
                        nc.vector.tensor_copy(qkT_sb[:, n, tsl], pq[:])

            def proj_v(tci):
                for tb in range(tci * 4, tci * 4 + 4):
                    pv = psP.tile([P, TCH], f32, tag="pp", name="pv")
                    pvv = pv[:, 0 : HPC * HD]
                    for kc in range(NKC):
                        nc.tensor.matmul(
                            pvv,
                            xT_sb[:, kc, tb * P : (tb + 1) * P],
                            wv_sb[:, kc, :],
                            start=(kc == 0),
                            stop=(kc == NKC - 1 and not has_bias),
                        )
                    if has_bias:
                        nc.tensor.matmul(
                            pvv,
                            mask_sb[0:1, P : 2 * P],
                            bv_sb[:1, :],
                            start=False,
                            stop=True,
                        )
                    nc.vector.tensor_copy(
                        v_sb[:, tb, :, 0:HD],
                        pvv.rearrange("p (h d) -> p h d", d=HD),
                    )

            def attn_chunk(pair, ci):
                qn, kn = pair, 2 + pair
                jmax = 4 * ci + 3
                po = [
                    psO.tile([HD + 1, TCH], f32, tag=f"po{hip}", name=f"po{hip}")
                    for hip in range(2)
                ]

                def qk_exp(j):
                    r = j - 4 * ci
                    col0 = P * r if r > 0 else 0
                    ps = psA.tile([P, 2, TCH], f32, tag="sc")
                    et = wpool.tile([P, 2, TCH], bf16, tag="expt")
                    for hip in range(2):
                        base = 64 * hip
                        nc.tensor.matmul(
                            ps[:, hip, col0:],
                            qkT_sb[base : base + 64, kn, j * P : (j + 1) * P],
                            qkT_sb[
                                base : base + 64,
                                qn,
                                ci * TCH + col0 : (ci + 1) * TCH,
                            ],
                            start=True,
                            stop=True,
                        )
                    nc.scalar.activation(
                        et[:, :, col0:], ps[:, :, col0:], Exp, scale=0.125
                    )
                    if r >= 0:
                        # diagonal block: zero the upper triangle of the
                        # leading 128 columns (bf16 2x DVE mode)
                        nc.vector.tensor_mul(
                            et[:, :, col0 : col0 + P],
                            et[:, :, col0 : col0 + P],
                            mask_sb[:, None, P : 2 * P].to_broadcast((P, 2, P)),
                        )
                    return et, col0

                def av(j, et, col0):
                    for hip in range(2):
                        h = 2 * pair + hip
                        nc.tensor.matmul(
                            po[hip][:, col0:],
                            v_sb[:, j, h, :],
                            et[:, hip, col0:],
                            start=(j == 0),
                            stop=(j == jmax),
                        )

                prev = None
                for j in range(jmax + 1):
                    cur = qk_exp(j)
                    if prev is not None:
                        av(j - 1, *prev)
                    prev = cur
                av(jmax, *prev)

                for hip in range(2):
                    h = 2 * pair + hip
                    # one DVE copy frees the PSUM bank immediately; the rest of
                    # the normalization runs from SBUF off the critical path
                    pou = opool.tile([HD + 1, TCH], f32, tag="pou")
                    nc.vector.tensor_copy(pou[:], po[hip][:])
                    recip = opool.tile([1, TCH], f32, tag="recip")
                    nc.vector.reciprocal(recip[:], pou[HD : HD + 1, :])
                    rbc = opool.tile([HD, TCH], f32, tag="rbc")
                    nc.gpsimd.partition_broadcast(rbc[:], recip[:], channels=HD)
                    osb = opool.tile([HD, TCH], f32, tag="osb")
                    nc.vector.tensor_mul(osb[:], pou[0:HD, :], rbc[:])
                    nc.sync.dma_start(
                        out_d[h, :, ci * TCH : (ci + 1) * TCH], osb[:]
                    )

            # ---------------- schedule ----------------
            # proj(tci=0) first; then per tq-chunk: attention for both pairs,
            # with proj(tci=ci+1) emitted between them so the PE fills exp
            # waits with projection matmuls.
            if do_proj and do_attn:
                proj_qk(0, 0)
                proj_qk(1, 0)
                proj_v(0)
                for ci in range(NTC):
                    attn_chunk(0, ci)
                    if ci + 1 < NTC:
                        proj_qk(0, ci + 1)
                        proj_qk(1, ci + 1)
                        proj_v(ci + 1)
                    attn_chunk(1, ci)
            elif do_proj:
                for tci in range(NTC):
                    proj_qk(0, tci)
                    proj_qk(1, tci)
                    proj_v(tci)
            elif do_attn:
                for pair in range(2):
                    for ci in range(NTC):
                        attn_chunk(pair, ci)
            if not external_io:
                nc.sync.dma_start(tiny_out_d[:], mask_sb[:, 0:P])

          if loops > 1:
            with tc.For_i(
                0, loops, 1, hint_engines=(mybir.EngineType.PE,)
            ) as _i:
                _body()
          else:
            _body()
    if not nc.is_finalized():
        nc.finalize()
    return nc


def make_in_maps(x, W, b):
    import ml_dtypes

    bf = ml_dtypes.bfloat16
    x = np.asarray(x, np.float32)
    W = np.asarray(W, np.float32)
    b = np.asarray(b, np.float32)
    tri = (np.arange(P)[:, None] <= np.arange(P)[None, :]).astype(bf)
    mask01 = np.concatenate([np.zeros((P, P), bf), tri], axis=1)
    in_maps = []
    for core in range(NCORES):
        bidx, g = divmod(core, GROUPS)
        xT = np.ascontiguousarray(x[bidx].T).astype(bf)
        cols = np.empty(4 * P, np.int64)
        for n in range(4):
            qk, pairi = divmod(n, 2)
            for p in range(P):
                hl = 2 * pairi + p // 64
                cols[n * P + p] = qk * D + (HPC * g + hl) * HD + (p % 64)
        sl = slice(2 * D + g * HPC * HD, 2 * D + (g + 1) * HPC * HD)
        in_maps.append(
            {
                "xT": xT,
                "onescol": np.ones((P, NTB, HPC, 1), bf),
                "wqk": np.ascontiguousarray(W[:, cols]).astype(bf),
                "bqk": np.ascontiguousarray(b[cols]),
                "wv": np.ascontiguousarray(W[:, sl]).astype(bf),
                "bv": np.ascontiguousarray(b[sl]).astype(bf),
                "mask": mask01,
            }
        )
    return in_maps


def assemble_output(per_core_out):
    O = np.empty((B, H, HD, T), np.float32)
    for core in range(NCORES):
        bidx, g = divmod(core, GROUPS)
        O[bidx, g * HPC : (g + 1) * HPC] = per_core_out[core]
    return np.ascontiguousarray(O.transpose(0, 3, 1, 2).reshape(B, T, H * HD))


def run(x, W_qkv, b_qkv, trace=False):
    from concourse.bass_utils import run_bass_kernel_spmd

    nc = build_nc(has_bias=bool(np.any(np.asarray(b_qkv))))
    in_maps = make_in_maps(x, W_qkv, b_qkv)
    res = run_bass_kernel_spmd(
        nc, in_maps, list(range(NCORES)), trace=trace
    )
    out = assemble_output([res.results[i]["out"] for i in range(NCORES)])
    return out, res


def kernel(x, W_qkv, b_qkv):
    out, _ = run(x, W_qkv, b_qkv, trace=False)
    return out
